# revision 16
# baseline (speedup 1.0000x reference)
"""Trainium2 Bass kernel for the per-row hypernetwork MLP.

Per row r:  h1 = relu(x[r]*W1.T + b1); h2 = relu(h1@W2.T + b2)
            w = h2@Wg.T + bg;          y[r] = w . g[r]

Sharding: pure data parallel over 8 NeuronCores.

Layout (per core): rows are packed on the host into "chunks" of
[128 partitions x 512 cols].  Each 32-lane strip of a chunk is a "block"
of 12 row-groups (= 2 supertiles of 6 groups x 512 rows):

  x stream  [.,128,512]: lane 32b+gamma (gamma<12) col j = x of row
            (chunk,b,gamma,j); lane 32b+31 = 1.0 (bias lane); rest 0
  g stream  [.,128,512]: lanes 0..12 of each strip = g[...,0],
            lanes 12..24 = g[...,1], rest zero
  y stream  [., 48,512]: lane 12b+gamma = y rows (compact)

Biases ride the constant ones-lane: M1's lhsT row 31 supplies b1 and a
"ones row" (h1[120]=1), M2's lhsT row 120 supplies b2 and per-group ones
rows (h2[21g+20]=1), M3's lhsT rows 21g+20 supply bg.  A1/A2 are then
pure relu ops.

Per supertile u (6 groups, gamma = 6u+g) of block b:
  M1: lhsT = L1u[32b:32b+32] [32,121] -> P1 [121,512]; A1: relu (DVE)
  M2: lhsT [121,126] block-diag W2 -> P2; A2: relu (ACT) -> H2 [126,512]
  M3: lhsT [126,128] (zero except cols of strip b): w0 -> lane 32b+g',
      w1 -> lane 32b+12+g'; all 8 supertiles of a chunk accumulate
      into ONE full [128,512] PSUM bank (start = first).
Per chunk:
  t  = P3 (.) gchunk        (one full-width DVE multiply)
  M4: ones-matmul pair-sum -> P4 [48,512] compact; ycopy ACT; DMA out.

All matmul operands are float32r (full-rate PE; ~fp32 accuracy).
"""

import numpy as np

import concourse.bass as bass
import concourse.bacc as bacc
import concourse.tile as tile
from concourse import mybir
from concourse import bass_utils

N_CORES = 8
H = 20
FREE = 512
CPD = 3                    # chunks per DMA group
CHUNK_ROWS = 48 * FREE     # 24576 rows per [128,512] chunk
F32 = mybir.dt.float32
F32R = mybir.dt.float32r
RELU = mybir.ActivationFunctionType.Relu
COPY = mybir.ActivationFunctionType.Copy
ALU_MAX = mybir.AluOpType.max

# Dev knobs (harness uses defaults).
TRACE = False
LAST_RESULT = None
USE_F32R = True
A1_PAT = "d"        # per-supertile engine for A1 relu: 'd'=DVE 'a'=ACT
A2_PAT = "a"        # per-supertile engine for A2 relu
YC_ENG = "act"      # ycopy engine
FINAL_F32 = True    # run t/M4 in fp32 (4x PE cost on M4 only)


def _build_nc(nchunk, use_f32r=True):
    assert nchunk % CPD == 0
    nc = bacc.Bacc("TRN2", target_bir_lowering=False, debug=False,
                   num_devices=N_CORES)
    DT = F32R if use_f32r else F32
    ngrp = nchunk // CPD
    x_d = nc.dram_tensor("x", [ngrp, 128, CPD, FREE], DT,
                         kind="ExternalInput")
    g_d = nc.dram_tensor("g", [ngrp, 128, CPD, FREE], F32,
                         kind="ExternalInput")
    l1_d = nc.dram_tensor("l1", [2, 128, 121], DT, kind="ExternalInput")
    l2_d = nc.dram_tensor("l2", [121, 126], DT, kind="ExternalInput")
    l3_d = nc.dram_tensor("l3", [8, 126, 128], DT, kind="ExternalInput")
    FDT = F32 if FINAL_F32 else DT
    ones_d = nc.dram_tensor("ones4", [128, 48], FDT, kind="ExternalInput")
    y_d = nc.dram_tensor("y", [ngrp, 48, CPD, FREE], F32,
                         kind="ExternalOutput")

    with tile.TileContext(nc) as tc:
        with (
            tc.tile_pool(name="const", bufs=1) as cp,
            tc.tile_pool(name="xin", bufs=2) as xp,
            tc.tile_pool(name="gin", bufs=2) as gp,
            tc.tile_pool(name="acts", bufs=3) as hp,
            tc.tile_pool(name="tmul", bufs=2) as tp,
            tc.tile_pool(name="yout", bufs=2) as yp,
            tc.tile_pool(name="psA", bufs=2, space="PSUM") as p1p,
            tc.tile_pool(name="psB", bufs=2, space="PSUM") as p2p,
            tc.tile_pool(name="psC", bufs=2, space="PSUM") as p3p,
            tc.tile_pool(name="psD", bufs=2, space="PSUM") as p4p,
        ):
            L1 = []
            for u in range(2):
                t_ = cp.tile([128, 121], DT, tag=f"l1{u}")
                nc.sync.dma_start(out=t_, in_=l1_d.ap()[u])
                L1.append(t_)
            L2 = cp.tile([121, 126], DT)
            nc.sync.dma_start(out=L2, in_=l2_d.ap())
            L3 = []
            for i in range(8):
                t_ = cp.tile([126, 128], DT, tag=f"l3{i}")
                nc.sync.dma_start(out=t_, in_=l3_d.ap()[i])
                L3.append(t_)
            ONES = cp.tile([128, 48], FDT)
            nc.sync.dma_start(out=ONES, in_=ones_d.ap())

            def relu_op(dst, srcp, pat, idx):
                if pat[idx % len(pat)] == "d":
                    nc.vector.tensor_scalar(dst, srcp, 0.0, None, ALU_MAX)
                else:
                    nc.scalar.activation(out=dst, in_=srcp, func=RELU)

            st = 0
            for d in range(ngrp):
                xc = xp.tile([128, CPD, FREE], DT, tag="xc")
                nc.sync.dma_start(out=xc, in_=x_d.ap()[d])
                gc = gp.tile([128, CPD, FREE], F32, tag="gc")
                nc.sync.dma_start(out=gc, in_=g_d.ap()[d])
                ysb = yp.tile([48, CPD, FREE], F32, tag="ysb")
                for cc in range(CPD):
                    p3 = p3p.tile([128, FREE], F32, tag="p3")
                    for b in range(4):
                        for u in range(2):
                            p1 = p1p.tile([121, FREE], F32, tag="p1")
                            nc.tensor.matmul(
                                p1, L1[u][32 * b:32 * b + 32],
                                xc[32 * b:32 * b + 32, cc, :],
                                start=True, stop=True,
                                tile_position=(32 * b, 0))
                            h1 = hp.tile([121, FREE], DT, tag="h1")
                            relu_op(h1[:], p1[:], A1_PAT, st)
                            p2 = p2p.tile([126, FREE], F32, tag="p2")
                            nc.tensor.matmul(p2, L2[:], h1[:],
                                             start=True, stop=True)
                            h2 = hp.tile([126, FREE], DT, tag="h2")
                            relu_op(h2[:], p2[:], A2_PAT, st)
                            nc.tensor.matmul(
                                p3, L3[2 * b + u][:], h2[:],
                                start=(b == 0 and u == 0),
                                stop=(b == 3 and u == 1))
                            st += 1
                    t_ = tp.tile([128, FREE], FDT, tag="t")
                    nc.vector.tensor_mul(t_[:], p3[:], gc[:, cc, :])
                    p4 = p4p.tile([48, FREE], F32, tag="p4")
                    nc.tensor.matmul(p4, ONES[:], t_[:],
                                     start=True, stop=True)
                    if YC_ENG == "dve":
                        nc.vector.tensor_copy(ysb[:, cc, :], p4[:])
                    else:
                        nc.scalar.activation(out=ysb[:, cc, :], in_=p4,
                                             func=COPY)
                nc.sync.dma_start(out=y_d.ap()[d], in_=ysb)
    nc.compile()
    return nc


def _prep_weights(W1, b1, W2, b2, Wg, bg):
    W1 = np.asarray(W1, np.float32).reshape(H, 1)
    b1 = np.asarray(b1, np.float32).reshape(H)
    W2 = np.asarray(W2, np.float32).reshape(H, H)
    b2 = np.asarray(b2, np.float32).reshape(H)
    Wg = np.asarray(Wg, np.float32).reshape(2, H)
    bg = np.asarray(bg, np.float32).reshape(2)

    L1 = np.zeros((2, 128, 121), np.float32)
    L2 = np.zeros((121, 126), np.float32)
    L3 = np.zeros((8, 126, 128), np.float32)
    ONES = np.zeros((128, 48), np.float32)
    for u in range(2):
        for b in range(4):
            for g in range(6):
                L1[u, 32 * b + 6 * u + g, 20 * g:20 * g + 20] = W1[:, 0]
            L1[u, 32 * b + 31, :] = 0.0
            for g in range(6):
                L1[u, 32 * b + 31, 20 * g:20 * g + 20] = b1
            L1[u, 32 * b + 31, 120] = 1.0
    for g in range(6):
        L2[20 * g:20 * g + 20, 21 * g:21 * g + 20] = W2.T
        L2[120, 21 * g:21 * g + 20] = b2
        L2[120, 21 * g + 20] = 1.0
    for b in range(4):
        for u in range(2):
            i = 2 * b + u
            for g in range(6):
                L3[i, 21 * g:21 * g + 20, 32 * b + 6 * u + g] = Wg[0]
                L3[i, 21 * g:21 * g + 20, 32 * b + 12 + 6 * u + g] = Wg[1]
                L3[i, 21 * g + 20, 32 * b + 6 * u + g] = bg[0]
                L3[i, 21 * g + 20, 32 * b + 12 + 6 * u + g] = bg[1]
    for b in range(4):
        for gm in range(12):
            ONES[32 * b + gm, 12 * b + gm] = 1.0
            ONES[32 * b + 12 + gm, 12 * b + gm] = 1.0
    return dict(l1=L1, l2=L2, l3=L3, ones4=ONES)


def _pack_streams(x, g0, g1, nchunk):
    """x, g0, g1: flat [npad] arrays -> packed DMA-layout arrays."""
    ngrp = nchunk // CPD
    xz = np.zeros((ngrp, CPD, 4, 32, FREE), np.float32)
    xz[:, :, :, :12, :] = x.reshape(ngrp, CPD, 4, 12, FREE)
    xz[:, :, :, 31, :] = 1.0          # bias lane
    xpk = np.ascontiguousarray(
        xz.transpose(0, 2, 3, 1, 4)).reshape(ngrp, 128, CPD, FREE)
    gz = np.zeros((ngrp, CPD, 4, 32, FREE), np.float32)
    gz[:, :, :, :12, :] = g0.reshape(ngrp, CPD, 4, 12, FREE)
    gz[:, :, :, 12:24, :] = g1.reshape(ngrp, CPD, 4, 12, FREE)
    gpk = np.ascontiguousarray(
        gz.transpose(0, 2, 3, 1, 4)).reshape(ngrp, 128, CPD, FREE)
    return xpk, gpk


def _unpack_y(y_dev, npad):
    """y_dev [ngrp,48,CPD,FREE] -> flat [npad]."""
    ngrp = y_dev.shape[0]
    return np.ascontiguousarray(
        y_dev.reshape(ngrp, 4, 12, CPD, FREE).transpose(0, 3, 1, 2, 4)
    ).reshape(-1)


_NC_CACHE = {}


def _get_nc(nchunk, use_f32r=True):
    key = (nchunk, use_f32r, A1_PAT, A2_PAT, YC_ENG, FINAL_F32)
    if key not in _NC_CACHE:
        _NC_CACHE[key] = _build_nc(nchunk, use_f32r)
    return _NC_CACHE[key]


def kernel(inputs_for_f, inputs_for_g, W1, b1, W2, b2, Wg, bg):
    x = np.asarray(inputs_for_f, np.float32).reshape(-1)
    g = np.asarray(inputs_for_g, np.float32)
    n = x.shape[0]
    per_core = -(-n // N_CORES)
    nchunk = -(-per_core // CHUNK_ROWS)
    nchunk = -(-nchunk // CPD) * CPD
    npad = nchunk * CHUNK_ROWS

    g0 = np.ascontiguousarray(g[:, 0])
    g1 = np.ascontiguousarray(g[:, 1])
    wmaps = _prep_weights(W1, b1, W2, b2, Wg, bg)
    nc = _get_nc(nchunk, USE_F32R)

    in_maps = []
    for i in range(N_CORES):
        lo = i * per_core
        hi = min(lo + per_core, n)
        cnt = hi - lo

        def pad(a):
            buf = np.zeros(npad, np.float32)
            buf[:cnt] = a[lo:hi]
            return buf

        xpk, gpk = _pack_streams(pad(x), pad(g0), pad(g1), nchunk)
        m = dict(x=xpk, g=gpk)
        m.update(wmaps)
        in_maps.append(m)

    try:
        res = bass_utils.run_bass_kernel_spmd(nc, in_maps,
                                              core_ids=list(range(N_CORES)),
                                              trace=TRACE)
    except ModuleNotFoundError:
        res = bass_utils.run_bass_kernel_spmd(nc, in_maps,
                                              core_ids=list(range(N_CORES)))
    global LAST_RESULT
    LAST_RESULT = res

    outs = []
    for i in range(N_CORES):
        lo = i * per_core
        hi = min(lo + per_core, n)
        outs.append(_unpack_y(res.results[i]["y"], npad)[:hi - lo])
    return np.concatenate(outs).reshape(n, 1)
